# revision 2
# baseline (speedup 1.0000x reference)
"""Trainium2 Bass kernel v4 — 8-batch-interleaved rows, non-T gathers.

All 8 batch samples share the gather index pattern, so rows store all 8
samples interleaved (8*64 bf16 = 1KB): one descriptor serves 8 samples,
cutting per-core stage-2 gather indices to 18432 at the cheap non-transpose
desc-gen rate (~1.5ns/idx vs ~8.6ns/idx for transpose mode).

Sharding: all cores u-shard stage 1 (2048 rows each) over host-interleaved
X8 [4096, 1KB] bf16; two chunked 8-rank AllGathers (pipelined behind
stage 1) assemble the full pooled table agout [16384, 1KB] on every core;
stage 2 v-shards 2048 rows/core: 9 non-T gathers per 1024-v chunk, DVE
depthwise (bf16), PE transpose + matmul per batch, v-major out
[2048, 8*32] f32 per core.
"""

import sys

import numpy as np

if "/opt/trn_rl_repo" not in sys.path:
    sys.path.insert(0, "/opt/trn_rl_repo")

import ml_dtypes
from contextlib import ExitStack

import concourse.tile as tile
from concourse import bacc, mybir
from concourse.bass_utils import run_bass_kernel_spmd
from concourse.masks import make_identity

B, VIN, C = 8, 4096, 64
VOUT, E, S, COUT = 16384, 49152, 9, 32
NCORES = 8
USH = VOUT // NCORES       # 2048 (stage-1 u-shard = stage-2 v-shard)
ROW = B * C                # 512 elems (1KB bf16)

SCRATCH = 65536
NQ = 4
GSZ = 1024
CU = 2048                  # stage-1 u-chunk (single AG)
CV = 1024                  # stage-2 v-chunk

F32 = mybir.dt.float32
BF16 = mybir.dt.bfloat16
I16 = mybir.dt.int16

_PROGRAM = None
_QCTR = [0]


def _gathers(nc, out_tile, in_ap, idx_sb, col0, total, elem):
    nsub = total // GSZ
    cols = GSZ // 16
    jg = GSZ // 128
    for g in range(nsub):
        nc.gpsimd.dma_gather(
            out_ap=out_tile[:, g * jg:(g + 1) * jg, :],
            in_ap=in_ap,
            idxs_ap=idx_sb[:, col0 + g * cols:col0 + (g + 1) * cols],
            num_idxs=GSZ,
            num_idxs_reg=GSZ,
            elem_size=elem,
            queue_num=_QCTR[0] % NQ,
        )
        _QCTR[0] += 1


def _build_program():
    _QCTR[0] = 0
    nc = bacc.Bacc("TRN2", target_bir_lowering=False, debug=False,
                   num_devices=NCORES, dynamic_dma_scratch_size=SCRATCH,
                   num_swdge_queues=NQ)

    x8 = nc.dram_tensor("x8", [VIN, ROW], BF16, kind="ExternalInput")
    idxcc = nc.dram_tensor("idxcc", [128, 3 * USH // 16], I16,
                           kind="ExternalInput")
    cvw = nc.dram_tensor("cvw", [128, 3 * USH // 128], BF16,
                         kind="ExternalInput")
    idxsp = nc.dram_tensor("idxsp", [128, S * USH // 16], I16,
                           kind="ExternalInput")
    wdrep = nc.dram_tensor("wdrep", [128, S * ROW], BF16,
                           kind="ExternalInput")
    wptbp = nc.dram_tensor("wptbp", [C + 1, COUT], F32, kind="ExternalInput")
    bdt = nc.dram_tensor("bdt", [C, 1], F32, kind="ExternalInput")
    out = nc.dram_tensor("out", [USH, B * COUT], F32, kind="ExternalOutput")

    NCH1 = USH // CU           # 2
    JU = CU // 128             # 8

    with tile.TileContext(nc) as tc, ExitStack() as ctx:
        const = ctx.enter_context(tc.tile_pool(name="const", bufs=1))
        dram = ctx.enter_context(tc.tile_pool(name="dram", bufs=1,
                                              space="DRAM"))

        agin = dram.tile([USH, ROW], BF16)
        agout = dram.tile([VOUT, ROW], BF16, addr_space="Shared")

        idxcc_sb = const.tile([128, 3 * USH // 16], I16)
        nc.sync.dma_start(idxcc_sb[:], idxcc.ap()[:])
        cvw_sb = const.tile([128, 3 * USH // 128], BF16)
        nc.sync.dma_start(cvw_sb[:], cvw.ap()[:])
        idxsp_sb = const.tile([128, S * USH // 16], I16)
        nc.sync.dma_start(idxsp_sb[:], idxsp.ap()[:])
        wd_sb = const.tile([128, S * ROW], BF16)
        nc.sync.dma_start(wd_sb[:], wdrep.ap()[:])
        wpt_sb = const.tile([C + 1, COUT], F32)
        nc.sync.dma_start(wpt_sb[:], wptbp.ap()[:])
        bd_sb = const.tile([C, 1], F32)
        nc.sync.dma_start(bd_sb[:], bdt.ap()[:])
        ident = const.tile([128, 128], BF16)
        make_identity(nc, ident[:])

        # ---- Stage 1 (2 chunks of 1024 u) + chunked AllGather ----
        agin_v = agin[:].rearrange("(ch jj p) d -> ch p jj d", p=128, jj=JU)
        with tc.tile_pool(name="s1", bufs=1) as s1:
            for ch in range(NCH1):
                xg = []
                for j in range(3):
                    g = s1.tile([128, JU, ROW], BF16, tag=f"xg{j}")
                    _gathers(nc, g, x8.ap()[:], idxcc_sb,
                             (ch * 3 + j) * (CU // 16), CU, ROW)
                    xg.append(g)

                def cvb(j, ch=ch):
                    a = (ch * 3 + j) * JU
                    return cvw_sb[:, a:a + JU, None].to_broadcast(
                        [128, JU, ROW])

                acc = s1.tile([128, JU, ROW], BF16, tag="acc")
                nc.vector.tensor_mul(acc[:], xg[0][:], cvb(0))
                m = s1.tile([128, JU, ROW], BF16, tag="m")
                nc.vector.tensor_mul(m[:], xg[1][:], cvb(1))
                nc.vector.tensor_add(acc[:], acc[:], m[:])
                nc.vector.tensor_mul(m[:], xg[2][:], cvb(2))
                nc.vector.tensor_add(acc[:], acc[:], m[:])
                nc.sync.dma_start(agin_v[ch], acc[:])
                nc.gpsimd.collective_compute(
                    "AllGather",
                    mybir.AluOpType.bypass,
                    replica_groups=[list(range(NCORES))],
                    ins=[agin[ch * CU:(ch + 1) * CU].opt()],
                    outs=[agout[ch * NCORES * CU:
                                (ch + 1) * NCORES * CU].opt()],
                )

        # ---- Stage 2: v-major, per-core shard 2048 v ----
        NCH2 = USH // CV       # 2
        JV = CV // 128         # 8
        out_v = out.ap()[:].rearrange(
            "(ch g q p) o -> ch g p q o", p=128, q=4, g=JV // 4)
        with tc.tile_pool(name="sp", bufs=3) as spp, \
             tc.tile_pool(name="s2m", bufs=2) as s2m, \
             tc.tile_pool(name="s2acc", bufs=2) as s2acc, \
             tc.tile_pool(name="dw", bufs=2) as dwp, \
             tc.tile_pool(name="osb", bufs=2) as outp, \
             tc.tile_pool(name="psT", bufs=2, space="PSUM") as psTp, \
             tc.tile_pool(name="ps2", bufs=2, space="PSUM") as ps2p:
            for ch in range(NCH2):
                acc = s2acc.tile([128, JV, ROW], BF16, tag="acc")
                for s in range(S):
                    sp = spp.tile([128, JV, ROW], BF16, tag="sp")
                    col0 = (ch * S + s) * (CV // 16)
                    _gathers(nc, sp, agout[:], idxsp_sb, col0, CV, ROW)
                    wdb = wd_sb[:, None, s * ROW:(s + 1) * ROW].to_broadcast(
                        [128, JV, ROW])
                    if s == 0:
                        nc.vector.tensor_mul(acc[:], sp[:], wdb)
                    else:
                        m2 = s2m.tile([128, JV, ROW], BF16, tag="m")
                        nc.vector.tensor_mul(m2[:], sp[:], wdb)
                        nc.vector.tensor_add(acc[:], acc[:], m2[:])
                for g in range(JV // 4):
                    for b in range(B):
                        psT = psTp.tile([C, 512], BF16)
                        for q in range(4):
                            nc.tensor.transpose(
                                psT[:, q * 128:(q + 1) * 128],
                                acc[:, g * 4 + q, b * C:(b + 1) * C],
                                ident[:],
                            )
                        dwT = dwp.tile([C + 1, 512], F32, tag="dwT")
                        nc.vector.memset(dwT[C:C + 1, :], 1.0)
                        nc.scalar.activation(
                            dwT[0:C, :], psT[:],
                            mybir.ActivationFunctionType.Identity,
                            bias=bd_sb[:],
                        )
                        ps2 = ps2p.tile([128, 4, COUT], F32)
                        for q in range(4):
                            nc.tensor.matmul(
                                ps2[:, q, :],
                                lhsT=dwT[:, q * 128:(q + 1) * 128],
                                rhs=wpt_sb[:],
                                start=True,
                                stop=True,
                            )
                        osb = outp.tile([128, 4, COUT], F32, tag="osb")
                        nc.scalar.activation(
                            osb[:], ps2[:], mybir.ActivationFunctionType.Relu)
                        nc.sync.dma_start(
                            out_v[ch, g][:, :, b * COUT:(b + 1) * COUT],
                            osb[:])

    nc.compile()
    return nc


def _wrap16(a):
    return np.tile(np.ascontiguousarray(a.reshape(-1, 16).T), (8, 1))


def _wrap_blocks(a, gsz):
    return np.concatenate(
        [_wrap16(a[g * gsz:(g + 1) * gsz]) for g in range(len(a) // gsz)],
        axis=1,
    )


def make_in_maps(np_inputs):
    x = np.asarray(np_inputs["x"], dtype=np.float32)
    tcol = np.asarray(np_inputs["trans_col"])
    tval = np.asarray(np_inputs["trans_value"], dtype=np.float32)
    rm = np.asarray(np_inputs["row_map"])
    idx = np.asarray(np_inputs["indices"])
    Wd = np.asarray(np_inputs["Wd"], dtype=np.float32)
    bd = np.asarray(np_inputs["bd"], dtype=np.float32)
    Wp = np.asarray(np_inputs["Wp"], dtype=np.float32)
    bp = np.asarray(np_inputs["bp"], dtype=np.float32)

    bf = ml_dtypes.bfloat16
    x8 = np.ascontiguousarray(
        x.transpose(1, 0, 2).reshape(VIN, ROW)).astype(bf)

    cc = tcol[rm].astype(np.int16)
    cv = tval[rm].astype(bf)

    wdrep = np.tile(Wd.T.reshape(S, 1, C), (1, B, 1)).reshape(1, S * ROW)
    wdrep = np.tile(wdrep, (128, 1)).astype(bf)
    wptbp = np.concatenate([Wp.T, bp[None, :]], axis=0).astype(np.float32)
    bdt = bd.reshape(C, 1).astype(np.float32)

    shared = dict(x8=x8, wdrep=wdrep, wptbp=wptbp, bdt=bdt)

    NCH1 = USH // CU
    # single AG: agout rows are rank-major 2048-shards in u order
    def rowp_of(u):
        return u

    in_maps = []
    for core in range(NCORES):
        u0 = core * USH
        ccs = cc[u0:u0 + USH]
        cvs = cv[u0:u0 + USH]
        idxcc = np.concatenate(
            [_wrap_blocks(ccs[ch * CU:(ch + 1) * CU, j], GSZ)
             for ch in range(NCH1) for j in range(3)], axis=1)
        cvw = np.concatenate(
            [np.ascontiguousarray(
                cvs[ch * CU:(ch + 1) * CU, j].reshape(-1, 128).T)
             for ch in range(NCH1) for j in range(3)], axis=1).astype(bf)

        v0 = core * USH
        vs = idx[v0:v0 + USH]                    # [USH, S]
        rowp = rowp_of(vs)
        assert rowp.max() <= 32767
        rowp = rowp.astype(np.int16)
        NCH2 = USH // CV
        idxsp = np.concatenate(
            [_wrap_blocks(rowp[ch * CV:(ch + 1) * CV, s], GSZ)
             for ch in range(NCH2) for s in range(S)], axis=1)
        in_maps.append({"idxcc": idxcc, "cvw": cvw, "idxsp": idxsp, **shared})
    return in_maps


def kernel(x, trans_row, trans_col, trans_value, row_map, indices,
           Wd, bd, Wp, bp):
    global _PROGRAM
    if _PROGRAM is None:
        _PROGRAM = _build_program()
    nc = _PROGRAM

    in_maps = make_in_maps(dict(x=x, trans_col=trans_col,
                                trans_value=trans_value, row_map=row_map,
                                indices=indices, Wd=Wd, bd=bd, Wp=Wp, bp=bp))
    res = run_bass_kernel_spmd(nc, in_maps, list(range(NCORES)))

    out = np.empty((B, VOUT, COUT), dtype=np.float32)
    for core in range(NCORES):
        o = res.results[core]["out"]             # [USH, B*COUT]
        v0 = core * USH
        out[:, v0:v0 + USH, :] = o.reshape(USH, B, COUT).transpose(1, 0, 2)
    return out


if __name__ == "__main__":
    _build_program()
    print("build ok")


# revision 3
# speedup vs baseline: 1.1307x; 1.1307x over previous
"""Trainium2 Bass kernel v4 — 8-batch-interleaved rows, non-T gathers.

All 8 batch samples share the gather index pattern, so rows store all 8
samples interleaved (8*64 bf16 = 1KB): one descriptor serves 8 samples,
cutting per-core stage-2 gather indices to 18432 at the cheap non-transpose
desc-gen rate (~1.5ns/idx vs ~8.6ns/idx for transpose mode).

Sharding: all cores u-shard stage 1 (2048 rows each) over host-interleaved
X8 [4096, 1KB] bf16; one 8-rank AllGather assembles the full pooled table
agout [16384, 1KB] bf16 on every core (rank-major concat = u order, so
stage-2 indices need no remap); stage 2 v-shards 2048 rows/core: 9 non-T
gathers per 1024-v chunk, DVE depthwise (bf16 2x mode), PE transpose +
matmul per batch against [Wp.T; bp] with bd via ACT bias, ACT relu,
v-major out [2048, 8*32] f32 per core; host splits batches at unshard.

Measured: 360-371us HW exec, rel err 0.0059 (vs 595us for the per-batch
baseline on the same measurement path). Timeline per core: stage-1 ~50us,
AllGather ~120us (serial), stage-2 gathers ~126us (drain-bound), PE tail.
"""

import sys

import numpy as np

if "/opt/trn_rl_repo" not in sys.path:
    sys.path.insert(0, "/opt/trn_rl_repo")

import ml_dtypes
from contextlib import ExitStack

import concourse.tile as tile
from concourse import bacc, mybir
from concourse.bass_utils import run_bass_kernel_spmd
from concourse.masks import make_identity

B, VIN, C = 8, 4096, 64
VOUT, E, S, COUT = 16384, 49152, 9, 32
NCORES = 8
USH = VOUT // NCORES       # 2048 (stage-1 u-shard = stage-2 v-shard)
ROW = B * C                # 512 elems (1KB bf16)

SCRATCH = 65536
NQ = 4
GSZ = 1024
CU = 2048                  # stage-1 u-chunk (single AG)
CV = 1024                  # stage-2 v-chunk

F32 = mybir.dt.float32
BF16 = mybir.dt.bfloat16
I16 = mybir.dt.int16

_PROGRAM = None
_QCTR = [0]


def _gathers(nc, out_tile, in_ap, idx_sb, col0, total, elem):
    nsub = total // GSZ
    cols = GSZ // 16
    jg = GSZ // 128
    for g in range(nsub):
        nc.gpsimd.dma_gather(
            out_ap=out_tile[:, g * jg:(g + 1) * jg, :],
            in_ap=in_ap,
            idxs_ap=idx_sb[:, col0 + g * cols:col0 + (g + 1) * cols],
            num_idxs=GSZ,
            num_idxs_reg=GSZ,
            elem_size=elem,
            queue_num=_QCTR[0] % NQ,
        )
        _QCTR[0] += 1


def _build_program():
    _QCTR[0] = 0
    nc = bacc.Bacc("TRN2", target_bir_lowering=False, debug=False,
                   num_devices=NCORES, dynamic_dma_scratch_size=SCRATCH,
                   num_swdge_queues=NQ)

    x8 = nc.dram_tensor("x8", [VIN, ROW], BF16, kind="ExternalInput")
    idxcc = nc.dram_tensor("idxcc", [128, 3 * USH // 16], I16,
                           kind="ExternalInput")
    cvw = nc.dram_tensor("cvw", [128, 3 * USH // 128], BF16,
                         kind="ExternalInput")
    idxsp = nc.dram_tensor("idxsp", [128, S * USH // 16], I16,
                           kind="ExternalInput")
    wdrep = nc.dram_tensor("wdrep", [128, S * ROW], BF16,
                           kind="ExternalInput")
    wptbp = nc.dram_tensor("wptbp", [C + 1, COUT], F32, kind="ExternalInput")
    bdt = nc.dram_tensor("bdt", [C, 1], F32, kind="ExternalInput")
    out = nc.dram_tensor("out", [USH, B * COUT], F32, kind="ExternalOutput")

    NCH1 = USH // CU           # 2
    JU = CU // 128             # 8

    with tile.TileContext(nc) as tc, ExitStack() as ctx:
        const = ctx.enter_context(tc.tile_pool(name="const", bufs=1))
        dram = ctx.enter_context(tc.tile_pool(name="dram", bufs=1,
                                              space="DRAM"))

        agin = dram.tile([USH, ROW], BF16)
        agout = dram.tile([VOUT, ROW], BF16, addr_space="Shared")

        idxcc_sb = const.tile([128, 3 * USH // 16], I16)
        nc.sync.dma_start(idxcc_sb[:], idxcc.ap()[:])
        cvw_sb = const.tile([128, 3 * USH // 128], BF16)
        nc.sync.dma_start(cvw_sb[:], cvw.ap()[:])
        idxsp_sb = const.tile([128, S * USH // 16], I16)
        nc.sync.dma_start(idxsp_sb[:], idxsp.ap()[:])
        wd_sb = const.tile([128, S * ROW], BF16)
        nc.sync.dma_start(wd_sb[:], wdrep.ap()[:])
        wpt_sb = const.tile([C + 1, COUT], F32)
        nc.sync.dma_start(wpt_sb[:], wptbp.ap()[:])
        bd_sb = const.tile([C, 1], F32)
        nc.sync.dma_start(bd_sb[:], bdt.ap()[:])
        ident = const.tile([128, 128], BF16)
        make_identity(nc, ident[:])

        # ---- Stage 1 (2 chunks of 1024 u) + chunked AllGather ----
        agin_v = agin[:].rearrange("(ch jj p) d -> ch p jj d", p=128, jj=JU)
        with tc.tile_pool(name="s1", bufs=1) as s1:
            for ch in range(NCH1):
                xg = []
                for j in range(3):
                    g = s1.tile([128, JU, ROW], BF16, tag=f"xg{j}")
                    _gathers(nc, g, x8.ap()[:], idxcc_sb,
                             (ch * 3 + j) * (CU // 16), CU, ROW)
                    xg.append(g)

                def cvb(j, ch=ch):
                    a = (ch * 3 + j) * JU
                    return cvw_sb[:, a:a + JU, None].to_broadcast(
                        [128, JU, ROW])

                acc = s1.tile([128, JU, ROW], BF16, tag="acc")
                nc.vector.tensor_mul(acc[:], xg[0][:], cvb(0))
                m = s1.tile([128, JU, ROW], BF16, tag="m")
                nc.vector.tensor_mul(m[:], xg[1][:], cvb(1))
                nc.vector.tensor_add(acc[:], acc[:], m[:])
                nc.vector.tensor_mul(m[:], xg[2][:], cvb(2))
                nc.vector.tensor_add(acc[:], acc[:], m[:])
                nc.sync.dma_start(agin_v[ch], acc[:])
                nc.gpsimd.collective_compute(
                    "AllGather",
                    mybir.AluOpType.bypass,
                    replica_groups=[list(range(NCORES))],
                    ins=[agin[ch * CU:(ch + 1) * CU].opt()],
                    outs=[agout[ch * NCORES * CU:
                                (ch + 1) * NCORES * CU].opt()],
                )

        # ---- Stage 2: v-major, per-core shard 2048 v ----
        NCH2 = USH // CV       # 2
        JV = CV // 128         # 8
        out_v = out.ap()[:].rearrange(
            "(ch g q p) o -> ch g p q o", p=128, q=4, g=JV // 4)
        with tc.tile_pool(name="sp", bufs=3) as spp, \
             tc.tile_pool(name="s2m", bufs=2) as s2m, \
             tc.tile_pool(name="s2acc", bufs=2) as s2acc, \
             tc.tile_pool(name="dw", bufs=2) as dwp, \
             tc.tile_pool(name="osb", bufs=2) as outp, \
             tc.tile_pool(name="psT", bufs=2, space="PSUM") as psTp, \
             tc.tile_pool(name="ps2", bufs=2, space="PSUM") as ps2p:
            for ch in range(NCH2):
                acc = s2acc.tile([128, JV, ROW], BF16, tag="acc")
                for s in range(S):
                    sp = spp.tile([128, JV, ROW], BF16, tag="sp")
                    col0 = (ch * S + s) * (CV // 16)
                    _gathers(nc, sp, agout[:], idxsp_sb, col0, CV, ROW)
                    wdb = wd_sb[:, None, s * ROW:(s + 1) * ROW].to_broadcast(
                        [128, JV, ROW])
                    if s == 0:
                        nc.vector.tensor_mul(acc[:], sp[:], wdb)
                    else:
                        m2 = s2m.tile([128, JV, ROW], BF16, tag="m")
                        nc.vector.tensor_mul(m2[:], sp[:], wdb)
                        nc.vector.tensor_add(acc[:], acc[:], m2[:])
                for g in range(JV // 4):
                    for b in range(B):
                        psT = psTp.tile([C, 512], BF16)
                        for q in range(4):
                            nc.tensor.transpose(
                                psT[:, q * 128:(q + 1) * 128],
                                acc[:, g * 4 + q, b * C:(b + 1) * C],
                                ident[:],
                            )
                        dwT = dwp.tile([C + 1, 512], F32, tag="dwT")
                        nc.vector.memset(dwT[C:C + 1, :], 1.0)
                        nc.scalar.activation(
                            dwT[0:C, :], psT[:],
                            mybir.ActivationFunctionType.Identity,
                            bias=bd_sb[:],
                        )
                        ps2 = ps2p.tile([128, 4, COUT], F32)
                        for q in range(4):
                            nc.tensor.matmul(
                                ps2[:, q, :],
                                lhsT=dwT[:, q * 128:(q + 1) * 128],
                                rhs=wpt_sb[:],
                                start=True,
                                stop=True,
                            )
                        osb = outp.tile([128, 4, COUT], F32, tag="osb")
                        nc.scalar.activation(
                            osb[:], ps2[:], mybir.ActivationFunctionType.Relu)
                        nc.sync.dma_start(
                            out_v[ch, g][:, :, b * COUT:(b + 1) * COUT],
                            osb[:])

    nc.compile()
    return nc


def _wrap16(a):
    return np.tile(np.ascontiguousarray(a.reshape(-1, 16).T), (8, 1))


def _wrap_blocks(a, gsz):
    return np.concatenate(
        [_wrap16(a[g * gsz:(g + 1) * gsz]) for g in range(len(a) // gsz)],
        axis=1,
    )


def make_in_maps(np_inputs):
    x = np.asarray(np_inputs["x"], dtype=np.float32)
    tcol = np.asarray(np_inputs["trans_col"])
    tval = np.asarray(np_inputs["trans_value"], dtype=np.float32)
    rm = np.asarray(np_inputs["row_map"])
    idx = np.asarray(np_inputs["indices"])
    Wd = np.asarray(np_inputs["Wd"], dtype=np.float32)
    bd = np.asarray(np_inputs["bd"], dtype=np.float32)
    Wp = np.asarray(np_inputs["Wp"], dtype=np.float32)
    bp = np.asarray(np_inputs["bp"], dtype=np.float32)

    bf = ml_dtypes.bfloat16
    x8 = np.ascontiguousarray(
        x.transpose(1, 0, 2).reshape(VIN, ROW)).astype(bf)

    cc = tcol[rm].astype(np.int16)
    cv = tval[rm].astype(bf)

    wdrep = np.tile(Wd.T.reshape(S, 1, C), (1, B, 1)).reshape(1, S * ROW)
    wdrep = np.tile(wdrep, (128, 1)).astype(bf)
    wptbp = np.concatenate([Wp.T, bp[None, :]], axis=0).astype(np.float32)
    bdt = bd.reshape(C, 1).astype(np.float32)

    shared = dict(x8=x8, wdrep=wdrep, wptbp=wptbp, bdt=bdt)

    NCH1 = USH // CU
    # single AG: agout rows are rank-major 2048-shards in u order
    def rowp_of(u):
        return u

    in_maps = []
    for core in range(NCORES):
        u0 = core * USH
        ccs = cc[u0:u0 + USH]
        cvs = cv[u0:u0 + USH]
        idxcc = np.concatenate(
            [_wrap_blocks(ccs[ch * CU:(ch + 1) * CU, j], GSZ)
             for ch in range(NCH1) for j in range(3)], axis=1)
        cvw = np.concatenate(
            [np.ascontiguousarray(
                cvs[ch * CU:(ch + 1) * CU, j].reshape(-1, 128).T)
             for ch in range(NCH1) for j in range(3)], axis=1).astype(bf)

        v0 = core * USH
        vs = idx[v0:v0 + USH]                    # [USH, S]
        rowp = rowp_of(vs)
        assert rowp.max() <= 32767
        rowp = rowp.astype(np.int16)
        NCH2 = USH // CV
        idxsp = np.concatenate(
            [_wrap_blocks(rowp[ch * CV:(ch + 1) * CV, s], GSZ)
             for ch in range(NCH2) for s in range(S)], axis=1)
        in_maps.append({"idxcc": idxcc, "cvw": cvw, "idxsp": idxsp, **shared})
    return in_maps


def kernel(x, trans_row, trans_col, trans_value, row_map, indices,
           Wd, bd, Wp, bp):
    global _PROGRAM
    if _PROGRAM is None:
        _PROGRAM = _build_program()
    nc = _PROGRAM

    in_maps = make_in_maps(dict(x=x, trans_col=trans_col,
                                trans_value=trans_value, row_map=row_map,
                                indices=indices, Wd=Wd, bd=bd, Wp=Wp, bp=bp))
    res = run_bass_kernel_spmd(nc, in_maps, list(range(NCORES)))

    out = np.empty((B, VOUT, COUT), dtype=np.float32)
    for core in range(NCORES):
        o = res.results[core]["out"]             # [USH, B*COUT]
        v0 = core * USH
        out[:, v0:v0 + USH, :] = o.reshape(USH, B, COUT).transpose(1, 0, 2)
    return out


if __name__ == "__main__":
    _build_program()
    print("build ok")


# revision 4
# speedup vs baseline: 1.1503x; 1.0173x over previous
"""Trainium2 Bass kernel v4 — 8-batch-interleaved rows, non-T gathers.

All 8 batch samples share the gather index pattern, so rows store all 8
samples interleaved (8*64 bf16 = 1KB): one descriptor serves 8 samples,
cutting per-core stage-2 gather indices to 18432 at the cheap non-transpose
desc-gen rate (~1.5ns/idx vs ~8.6ns/idx for transpose mode).

Sharding: all cores u-shard stage 1 (2048 rows each) over host-interleaved
X8 [4096, 1KB] bf16; two chunked 8-rank AllGathers (pipelined behind
stage 1) assemble the full pooled table agout [16384, 1KB] on every core;
stage 2 v-shards 2048 rows/core: 9 non-T gathers per 1024-v chunk, DVE
depthwise (bf16), PE transpose + matmul per batch, v-major out
[2048, 8*32] f32 per core.
"""

import sys

import numpy as np

if "/opt/trn_rl_repo" not in sys.path:
    sys.path.insert(0, "/opt/trn_rl_repo")

import ml_dtypes
from contextlib import ExitStack

import concourse.tile as tile
from concourse import bacc, mybir
from concourse.bass_utils import run_bass_kernel_spmd
from concourse.masks import make_identity

B, VIN, C = 8, 4096, 64
VOUT, E, S, COUT = 16384, 49152, 9, 32
NCORES = 8
USH = VOUT // NCORES       # 2048 (stage-1 u-shard = stage-2 v-shard)
ROW = B * C                # 512 elems (1KB bf16)

SCRATCH = 65536
NQ = 4
GSZ = 1024
CU = 1024                  # stage-1 u-chunk (2 chunks, single AG)
CV = 512                   # stage-2 v-chunk (short PE tail)

F32 = mybir.dt.float32
BF16 = mybir.dt.bfloat16
I16 = mybir.dt.int16

_PROGRAM = None
_QCTR = [0]


def _gathers(nc, out_tile, in_ap, idx_sb, col0, total, elem):
    gsz = min(GSZ, total)
    nsub = total // gsz
    cols = gsz // 16
    jg = gsz // 128
    for g in range(nsub):
        nc.gpsimd.dma_gather(
            out_ap=out_tile[:, g * jg:(g + 1) * jg, :],
            in_ap=in_ap,
            idxs_ap=idx_sb[:, col0 + g * cols:col0 + (g + 1) * cols],
            num_idxs=gsz,
            num_idxs_reg=gsz,
            elem_size=elem,
            queue_num=_QCTR[0] % NQ,
        )
        _QCTR[0] += 1


def _build_program():
    _QCTR[0] = 0
    nc = bacc.Bacc("TRN2", target_bir_lowering=False, debug=False,
                   num_devices=NCORES, dynamic_dma_scratch_size=SCRATCH,
                   num_swdge_queues=NQ)

    x8 = nc.dram_tensor("x8", [VIN, ROW], BF16, kind="ExternalInput")
    idxcc = nc.dram_tensor("idxcc", [128, 3 * USH // 16], I16,
                           kind="ExternalInput")
    cvw = nc.dram_tensor("cvw", [128, 3 * USH // 128], BF16,
                         kind="ExternalInput")
    idxsp = nc.dram_tensor("idxsp", [128, S * USH // 16], I16,
                           kind="ExternalInput")
    wdrep = nc.dram_tensor("wdrep", [128, S * ROW], BF16,
                           kind="ExternalInput")
    wptbp = nc.dram_tensor("wptbp", [C + 1, COUT], F32, kind="ExternalInput")
    bdt = nc.dram_tensor("bdt", [C, 1], F32, kind="ExternalInput")
    out = nc.dram_tensor("out", [USH, B * COUT], F32, kind="ExternalOutput")

    NCH1 = USH // CU           # 2
    JU = CU // 128             # 8

    with tile.TileContext(nc) as tc, ExitStack() as ctx:
        const = ctx.enter_context(tc.tile_pool(name="const", bufs=1))
        dram = ctx.enter_context(tc.tile_pool(name="dram", bufs=1,
                                              space="DRAM"))

        agin = dram.tile([USH, ROW], BF16)
        agout = dram.tile([VOUT, ROW], BF16, addr_space="Shared")

        idxcc_sb = const.tile([128, 3 * USH // 16], I16)
        nc.sync.dma_start(idxcc_sb[:], idxcc.ap()[:])
        cvw_sb = const.tile([128, 3 * USH // 128], BF16)
        nc.sync.dma_start(cvw_sb[:], cvw.ap()[:])
        idxsp_sb = const.tile([128, S * USH // 16], I16)
        nc.sync.dma_start(idxsp_sb[:], idxsp.ap()[:])
        wd_sb = const.tile([128, S * ROW], BF16)
        nc.sync.dma_start(wd_sb[:], wdrep.ap()[:])
        wpt_sb = const.tile([C + 1, COUT], F32)
        nc.sync.dma_start(wpt_sb[:], wptbp.ap()[:])
        bd_sb = const.tile([C, 1], F32)
        nc.sync.dma_start(bd_sb[:], bdt.ap()[:])
        ident = const.tile([128, 128], BF16)
        make_identity(nc, ident[:])

        # ---- Stage 1 (2 chunks of 1024 u) + chunked AllGather ----
        agin_v = agin[:].rearrange("(ch jj p) d -> ch p jj d", p=128, jj=JU)
        with tc.tile_pool(name="s1", bufs=1) as s1:
            for ch in range(NCH1):
                xg = []
                for j in range(3):
                    g = s1.tile([128, JU, ROW], BF16, tag=f"xg{j}")
                    _gathers(nc, g, x8.ap()[:], idxcc_sb,
                             (ch * 3 + j) * (CU // 16), CU, ROW)
                    xg.append(g)

                def cvb(j, ch=ch):
                    a = (ch * 3 + j) * JU
                    return cvw_sb[:, a:a + JU, None].to_broadcast(
                        [128, JU, ROW])

                acc = s1.tile([128, JU, ROW], BF16, tag="acc")
                nc.vector.tensor_mul(acc[:], xg[0][:], cvb(0))
                m = s1.tile([128, JU, ROW], BF16, tag="m")
                nc.vector.tensor_mul(m[:], xg[1][:], cvb(1))
                nc.vector.tensor_add(acc[:], acc[:], m[:])
                nc.vector.tensor_mul(m[:], xg[2][:], cvb(2))
                nc.vector.tensor_add(acc[:], acc[:], m[:])
                nc.sync.dma_start(agin_v[ch], acc[:])
        nc.gpsimd.collective_compute(
            "AllGather",
            mybir.AluOpType.bypass,
            replica_groups=[list(range(NCORES))],
            ins=[agin.opt()],
            outs=[agout.opt()],
        )

        # ---- Stage 2: v-major, per-core shard 2048 v ----
        NCH2 = USH // CV       # 2
        JV = CV // 128         # 8
        out_v = out.ap()[:].rearrange(
            "(ch g q p) o -> ch g p q o", p=128, q=4, g=JV // 4)
        with tc.tile_pool(name="sp", bufs=3) as spp, \
             tc.tile_pool(name="s2m", bufs=2) as s2m, \
             tc.tile_pool(name="s2acc", bufs=2) as s2acc, \
             tc.tile_pool(name="dw", bufs=1) as dwp, \
             tc.tile_pool(name="osb", bufs=2) as outp, \
             tc.tile_pool(name="psT", bufs=2, space="PSUM") as psTp, \
             tc.tile_pool(name="ps2", bufs=2, space="PSUM") as ps2p:
            dwts = []
            for k in range(2):
                t = dwp.tile([C + 1, 512], F32, tag=f"dwT{k}")
                nc.vector.memset(t[C:C + 1, :], 1.0)
                dwts.append(t)
            dwctr = [0]
            for ch in range(NCH2):
                acc = s2acc.tile([128, JV, ROW], BF16, tag="acc")
                for s in range(S):
                    sp = spp.tile([128, JV, ROW], BF16, tag="sp")
                    col0 = (ch * S + s) * (CV // 16)
                    _gathers(nc, sp, agout[:], idxsp_sb, col0, CV, ROW)
                    wdb = wd_sb[:, None, s * ROW:(s + 1) * ROW].to_broadcast(
                        [128, JV, ROW])
                    if s == 0:
                        nc.vector.tensor_mul(acc[:], sp[:], wdb)
                    else:
                        m2 = s2m.tile([128, JV, ROW], BF16, tag="m")
                        nc.vector.tensor_mul(m2[:], sp[:], wdb)
                        nc.vector.tensor_add(acc[:], acc[:], m2[:])
                for g in range(JV // 4):
                    for b in range(B):
                        psT = psTp.tile([C, 512], BF16)
                        for q in range(4):
                            nc.tensor.transpose(
                                psT[:, q * 128:(q + 1) * 128],
                                acc[:, g * 4 + q, b * C:(b + 1) * C],
                                ident[:],
                            )
                        dwT = dwts[dwctr[0] % 2]
                        dwctr[0] += 1
                        nc.scalar.activation(
                            dwT[0:C, :], psT[:],
                            mybir.ActivationFunctionType.Identity,
                            bias=bd_sb[:],
                        )
                        ps2 = ps2p.tile([128, 4, COUT], F32)
                        for q in range(4):
                            nc.tensor.matmul(
                                ps2[:, q, :],
                                lhsT=dwT[:, q * 128:(q + 1) * 128],
                                rhs=wpt_sb[:],
                                start=True,
                                stop=True,
                            )
                        osb = outp.tile([128, 4, COUT], F32, tag="osb")
                        nc.scalar.activation(
                            osb[:], ps2[:], mybir.ActivationFunctionType.Relu)
                        nc.sync.dma_start(
                            out_v[ch, g][:, :, b * COUT:(b + 1) * COUT],
                            osb[:])

    nc.compile()
    return nc


def _wrap16(a):
    return np.tile(np.ascontiguousarray(a.reshape(-1, 16).T), (8, 1))


def _wrap_blocks(a, gsz):
    return np.concatenate(
        [_wrap16(a[g * gsz:(g + 1) * gsz]) for g in range(len(a) // gsz)],
        axis=1,
    )


def make_in_maps(np_inputs):
    x = np.asarray(np_inputs["x"], dtype=np.float32)
    tcol = np.asarray(np_inputs["trans_col"])
    tval = np.asarray(np_inputs["trans_value"], dtype=np.float32)
    rm = np.asarray(np_inputs["row_map"])
    idx = np.asarray(np_inputs["indices"])
    Wd = np.asarray(np_inputs["Wd"], dtype=np.float32)
    bd = np.asarray(np_inputs["bd"], dtype=np.float32)
    Wp = np.asarray(np_inputs["Wp"], dtype=np.float32)
    bp = np.asarray(np_inputs["bp"], dtype=np.float32)

    bf = ml_dtypes.bfloat16
    x8 = np.ascontiguousarray(
        x.transpose(1, 0, 2).reshape(VIN, ROW)).astype(bf)

    cc = tcol[rm].astype(np.int16)
    cv = tval[rm].astype(bf)

    wdrep = np.tile(Wd.T.reshape(S, 1, C), (1, B, 1)).reshape(1, S * ROW)
    wdrep = np.tile(wdrep, (128, 1)).astype(bf)
    wptbp = np.concatenate([Wp.T, bp[None, :]], axis=0).astype(np.float32)
    bdt = bd.reshape(C, 1).astype(np.float32)

    shared = dict(x8=x8, wdrep=wdrep, wptbp=wptbp, bdt=bdt)

    NCH1 = USH // CU
    # single AG: agout rows are rank-major 2048-shards in u order
    def rowp_of(u):
        return u

    in_maps = []
    for core in range(NCORES):
        u0 = core * USH
        ccs = cc[u0:u0 + USH]
        cvs = cv[u0:u0 + USH]
        idxcc = np.concatenate(
            [_wrap_blocks(ccs[ch * CU:(ch + 1) * CU, j], min(GSZ, CU))
             for ch in range(NCH1) for j in range(3)], axis=1)
        cvw = np.concatenate(
            [np.ascontiguousarray(
                cvs[ch * CU:(ch + 1) * CU, j].reshape(-1, 128).T)
             for ch in range(NCH1) for j in range(3)], axis=1).astype(bf)

        v0 = core * USH
        vs = idx[v0:v0 + USH]                    # [USH, S]
        rowp = rowp_of(vs)
        assert rowp.max() <= 32767
        rowp = rowp.astype(np.int16)
        NCH2 = USH // CV
        idxsp = np.concatenate(
            [_wrap_blocks(rowp[ch * CV:(ch + 1) * CV, s], min(GSZ, CV))
             for ch in range(NCH2) for s in range(S)], axis=1)
        in_maps.append({"idxcc": idxcc, "cvw": cvw, "idxsp": idxsp, **shared})
    return in_maps


def kernel(x, trans_row, trans_col, trans_value, row_map, indices,
           Wd, bd, Wp, bp):
    global _PROGRAM
    if _PROGRAM is None:
        _PROGRAM = _build_program()
    nc = _PROGRAM

    in_maps = make_in_maps(dict(x=x, trans_col=trans_col,
                                trans_value=trans_value, row_map=row_map,
                                indices=indices, Wd=Wd, bd=bd, Wp=Wp, bp=bp))
    res = run_bass_kernel_spmd(nc, in_maps, list(range(NCORES)))

    out = np.empty((B, VOUT, COUT), dtype=np.float32)
    for core in range(NCORES):
        o = res.results[core]["out"]             # [USH, B*COUT]
        v0 = core * USH
        out[:, v0:v0 + USH, :] = o.reshape(USH, B, COUT).transpose(1, 0, 2)
    return out


if __name__ == "__main__":
    _build_program()
    print("build ok")


# revision 5
# speedup vs baseline: 1.1790x; 1.0250x over previous
"""Trainium2 Bass kernel v4 — 8-batch-interleaved rows, non-T gathers.

All 8 batch samples share the gather index pattern, so rows store all 8
samples interleaved (8*64 bf16 = 1KB): one descriptor serves 8 samples,
cutting per-core stage-2 gather indices to 18432 at the cheap non-transpose
desc-gen rate (~1.5ns/idx vs ~8.6ns/idx for transpose mode).

Sharding: all cores u-shard stage 1 (2048 rows each) over host-interleaved
X8 [4096, 1KB] bf16; two chunked 8-rank AllGathers (pipelined behind
stage 1) assemble the full pooled table agout [16384, 1KB] on every core;
stage 2 v-shards 2048 rows/core: 9 non-T gathers per 1024-v chunk, DVE
depthwise (bf16), PE transpose + matmul per batch, v-major out
[2048, 8*32] f32 per core.
"""

import sys

import numpy as np

if "/opt/trn_rl_repo" not in sys.path:
    sys.path.insert(0, "/opt/trn_rl_repo")

import ml_dtypes
from contextlib import ExitStack

import concourse.tile as tile
from concourse import bacc, mybir
from concourse.bass_utils import run_bass_kernel_spmd
from concourse.masks import make_identity

B, VIN, C = 8, 4096, 64
VOUT, E, S, COUT = 16384, 49152, 9, 32
NCORES = 8
USH = VOUT // NCORES       # 2048 (stage-1 u-shard = stage-2 v-shard)
GRP = 4                    # batches per table half
ROW = GRP * C              # 256 elems (512B bf16)

SCRATCH = 65536
NQ = 4
GSZ = 1024
CU = 1024                  # stage-1 u-chunk (2 chunks, single AG)
CV = 512                   # stage-2 v-chunk (short PE tail)

F32 = mybir.dt.float32
BF16 = mybir.dt.bfloat16
I16 = mybir.dt.int16

_PROGRAM = None
_QCTR = [0]


def _gathers(nc, out_tile, in_ap, idx_sb, col0, total, elem):
    gsz = min(GSZ, total)
    nsub = total // gsz
    cols = gsz // 16
    jg = gsz // 128
    for g in range(nsub):
        nc.gpsimd.dma_gather(
            out_ap=out_tile[:, g * jg:(g + 1) * jg, :],
            in_ap=in_ap,
            idxs_ap=idx_sb[:, col0 + g * cols:col0 + (g + 1) * cols],
            num_idxs=gsz,
            num_idxs_reg=gsz,
            elem_size=elem,
            queue_num=_QCTR[0] % NQ,
        )
        _QCTR[0] += 1


def _build_program():
    _QCTR[0] = 0
    nc = bacc.Bacc("TRN2", target_bir_lowering=False, debug=False,
                   num_devices=NCORES, dynamic_dma_scratch_size=SCRATCH,
                   num_swdge_queues=NQ)

    xa = nc.dram_tensor("xa", [VIN, ROW], BF16, kind="ExternalInput")
    xb = nc.dram_tensor("xb", [VIN, ROW], BF16, kind="ExternalInput")
    idxcc = nc.dram_tensor("idxcc", [128, 3 * USH // 16], I16,
                           kind="ExternalInput")
    cvw = nc.dram_tensor("cvw", [128, 3 * USH // 128], BF16,
                         kind="ExternalInput")
    idxsp = nc.dram_tensor("idxsp", [128, S * USH // 16], I16,
                           kind="ExternalInput")
    wdrep = nc.dram_tensor("wdrep", [128, S * ROW], BF16,
                           kind="ExternalInput")
    wptbp = nc.dram_tensor("wptbp", [C + 1, COUT], F32, kind="ExternalInput")
    bdt = nc.dram_tensor("bdt", [C, 1], F32, kind="ExternalInput")
    out = nc.dram_tensor("out", [USH, B * COUT], F32,
                         kind="ExternalOutput")

    NCH1 = USH // CU           # 2
    JU = CU // 128             # 8

    with tile.TileContext(nc) as tc, ExitStack() as ctx:
        const = ctx.enter_context(tc.tile_pool(name="const", bufs=1))
        dram = ctx.enter_context(tc.tile_pool(name="dram", bufs=1,
                                              space="DRAM"))

        agins = [dram.tile([USH, ROW], BF16, name=f"agin{k}")
                 for k in range(2)]
        agouts = [dram.tile([VOUT, ROW], BF16, addr_space="Shared",
                            name=f"agout{k}") for k in range(2)]

        idxcc_sb = const.tile([128, 3 * USH // 16], I16)
        nc.sync.dma_start(idxcc_sb[:], idxcc.ap()[:])
        cvw_sb = const.tile([128, 3 * USH // 128], BF16)
        nc.sync.dma_start(cvw_sb[:], cvw.ap()[:])
        idxsp_sb = const.tile([128, S * USH // 16], I16)
        nc.sync.dma_start(idxsp_sb[:], idxsp.ap()[:])
        wd_sb = const.tile([128, S * ROW], BF16)
        nc.sync.dma_start(wd_sb[:], wdrep.ap()[:])
        wpt_sb = const.tile([C + 1, COUT], F32)
        nc.sync.dma_start(wpt_sb[:], wptbp.ap()[:])
        bd_sb = const.tile([C, 1], F32)
        nc.sync.dma_start(bd_sb[:], bdt.ap()[:])
        ident = const.tile([128, 128], BF16)
        make_identity(nc, ident[:])

        # ---- Stage 1: per batch-half, AG fires as soon as its half done --
        with tc.tile_pool(name="s1", bufs=2) as s1:
            for grp, xsrc in enumerate((xa, xb)):
                agin_v = agins[grp][:].rearrange(
                    "(ch jj p) d -> ch p jj d", p=128, jj=JU)
                for ch in range(NCH1):
                    xg = []
                    for j in range(3):
                        g = s1.tile([128, JU, ROW], BF16, tag=f"xg{j}")
                        _gathers(nc, g, xsrc.ap()[:], idxcc_sb,
                                 (ch * 3 + j) * (CU // 16), CU, ROW)
                        xg.append(g)

                    def cvb(j, ch=ch):
                        a = (ch * 3 + j) * JU
                        return cvw_sb[:, a:a + JU, None].to_broadcast(
                            [128, JU, ROW])

                    acc = s1.tile([128, JU, ROW], BF16, tag="acc")
                    nc.vector.tensor_mul(acc[:], xg[0][:], cvb(0))
                    m = s1.tile([128, JU, ROW], BF16, tag="m")
                    nc.vector.tensor_mul(m[:], xg[1][:], cvb(1))
                    nc.vector.tensor_add(acc[:], acc[:], m[:])
                    nc.vector.tensor_mul(m[:], xg[2][:], cvb(2))
                    nc.vector.tensor_add(acc[:], acc[:], m[:])
                    nc.sync.dma_start(agin_v[ch], acc[:])
                nc.gpsimd.collective_compute(
                    "AllGather",
                    mybir.AluOpType.bypass,
                    replica_groups=[list(range(NCORES))],
                    ins=[agins[grp].opt()],
                    outs=[agouts[grp].opt()],
                )

        # ---- Stage 2: v-major, per-core shard 2048 v ----
        NCH2 = USH // CV       # 2
        JV = CV // 128         # 8
        out_v = out.ap()[:].rearrange(
            "(ch g q p) o -> ch g p q o", p=128, q=4, g=JV // 4)
        with tc.tile_pool(name="sp", bufs=4) as spp, \
             tc.tile_pool(name="s2m", bufs=2) as s2m, \
             tc.tile_pool(name="s2acc", bufs=2) as s2acc, \
             tc.tile_pool(name="dw", bufs=1) as dwp, \
             tc.tile_pool(name="osb", bufs=2) as outp, \
             tc.tile_pool(name="psT", bufs=2, space="PSUM") as psTp, \
             tc.tile_pool(name="ps2", bufs=2, space="PSUM") as ps2p:
            dwts = []
            for k in range(2):
                t = dwp.tile([C + 1, 512], F32, tag=f"dwT{k}")
                nc.vector.memset(t[C:C + 1, :], 1.0)
                dwts.append(t)
            dwctr = [0]
            for grp in range(2):
              for ch in range(NCH2):
                acc = s2acc.tile([128, JV, ROW], BF16, tag="acc")
                for s in range(S):
                    sp = spp.tile([128, JV, ROW], BF16, tag="sp")
                    col0 = (ch * S + s) * (CV // 16)
                    _gathers(nc, sp, agouts[grp][:], idxsp_sb, col0, CV, ROW)
                    wdb = wd_sb[:, None, s * ROW:(s + 1) * ROW].to_broadcast(
                        [128, JV, ROW])
                    if s == 0:
                        nc.vector.tensor_mul(acc[:], sp[:], wdb)
                    else:
                        m2 = s2m.tile([128, JV, ROW], BF16, tag="m")
                        nc.vector.tensor_mul(m2[:], sp[:], wdb)
                        nc.vector.tensor_add(acc[:], acc[:], m2[:])
                for g in range(JV // 4):
                    for b in range(GRP):
                        psT = psTp.tile([C, 512], BF16)
                        for q in range(4):
                            nc.tensor.transpose(
                                psT[:, q * 128:(q + 1) * 128],
                                acc[:, g * 4 + q, b * C:(b + 1) * C],
                                ident[:],
                            )
                        dwT = dwts[dwctr[0] % 2]
                        dwctr[0] += 1
                        nc.scalar.activation(
                            dwT[0:C, :], psT[:],
                            mybir.ActivationFunctionType.Identity,
                            bias=bd_sb[:],
                        )
                        ps2 = ps2p.tile([128, 4, COUT], F32)
                        for q in range(4):
                            nc.tensor.matmul(
                                ps2[:, q, :],
                                lhsT=dwT[:, q * 128:(q + 1) * 128],
                                rhs=wpt_sb[:],
                                start=True,
                                stop=True,
                            )
                        osb = outp.tile([128, 4, COUT], F32, tag="osb")
                        nc.scalar.activation(
                            osb[:], ps2[:], mybir.ActivationFunctionType.Relu)
                        nc.sync.dma_start(
                            out_v[ch, g][:, :,
                                         grp * GRP * COUT + b * COUT:
                                         grp * GRP * COUT + (b + 1) * COUT],
                            osb[:])

    nc.compile()
    return nc


def _wrap16(a):
    return np.tile(np.ascontiguousarray(a.reshape(-1, 16).T), (8, 1))


def _wrap_blocks(a, gsz):
    return np.concatenate(
        [_wrap16(a[g * gsz:(g + 1) * gsz]) for g in range(len(a) // gsz)],
        axis=1,
    )


def make_in_maps(np_inputs):
    x = np.asarray(np_inputs["x"], dtype=np.float32)
    tcol = np.asarray(np_inputs["trans_col"])
    tval = np.asarray(np_inputs["trans_value"], dtype=np.float32)
    rm = np.asarray(np_inputs["row_map"])
    idx = np.asarray(np_inputs["indices"])
    Wd = np.asarray(np_inputs["Wd"], dtype=np.float32)
    bd = np.asarray(np_inputs["bd"], dtype=np.float32)
    Wp = np.asarray(np_inputs["Wp"], dtype=np.float32)
    bp = np.asarray(np_inputs["bp"], dtype=np.float32)

    bf = ml_dtypes.bfloat16
    xa = np.ascontiguousarray(
        x[0:GRP].transpose(1, 0, 2).reshape(VIN, ROW)).astype(bf)
    xbv = np.ascontiguousarray(
        x[GRP:B].transpose(1, 0, 2).reshape(VIN, ROW)).astype(bf)

    cc = tcol[rm].astype(np.int16)
    cv = tval[rm].astype(bf)

    wdrep = np.tile(Wd.T.reshape(S, 1, C), (1, GRP, 1)).reshape(1, S * ROW)
    wdrep = np.tile(wdrep, (128, 1)).astype(bf)
    wptbp = np.concatenate([Wp.T, bp[None, :]], axis=0).astype(np.float32)
    bdt = bd.reshape(C, 1).astype(np.float32)

    shared = dict(xa=xa, xb=xbv, wdrep=wdrep, wptbp=wptbp, bdt=bdt)

    NCH1 = USH // CU
    # single AG: agout rows are rank-major 2048-shards in u order
    def rowp_of(u):
        return u

    in_maps = []
    for core in range(NCORES):
        u0 = core * USH
        ccs = cc[u0:u0 + USH]
        cvs = cv[u0:u0 + USH]
        idxcc = np.concatenate(
            [_wrap_blocks(ccs[ch * CU:(ch + 1) * CU, j], min(GSZ, CU))
             for ch in range(NCH1) for j in range(3)], axis=1)
        cvw = np.concatenate(
            [np.ascontiguousarray(
                cvs[ch * CU:(ch + 1) * CU, j].reshape(-1, 128).T)
             for ch in range(NCH1) for j in range(3)], axis=1).astype(bf)

        v0 = core * USH
        vs = idx[v0:v0 + USH]                    # [USH, S]
        rowp = rowp_of(vs)
        assert rowp.max() <= 32767
        rowp = rowp.astype(np.int16)
        NCH2 = USH // CV
        idxsp = np.concatenate(
            [_wrap_blocks(rowp[ch * CV:(ch + 1) * CV, s], min(GSZ, CV))
             for ch in range(NCH2) for s in range(S)], axis=1)
        in_maps.append({"idxcc": idxcc, "cvw": cvw, "idxsp": idxsp, **shared})
    return in_maps


def kernel(x, trans_row, trans_col, trans_value, row_map, indices,
           Wd, bd, Wp, bp):
    global _PROGRAM
    if _PROGRAM is None:
        _PROGRAM = _build_program()
    nc = _PROGRAM

    in_maps = make_in_maps(dict(x=x, trans_col=trans_col,
                                trans_value=trans_value, row_map=row_map,
                                indices=indices, Wd=Wd, bd=bd, Wp=Wp, bp=bp))
    res = run_bass_kernel_spmd(nc, in_maps, list(range(NCORES)))

    out = np.empty((B, VOUT, COUT), dtype=np.float32)
    for core in range(NCORES):
        o = res.results[core]["out"]             # [USH, B*COUT]
        v0 = core * USH
        out[:, v0:v0 + USH, :] = o.reshape(USH, B, COUT).transpose(1, 0, 2)
    return out


if __name__ == "__main__":
    _build_program()
    print("build ok")
